# revision 1
# baseline (speedup 1.0000x reference)
"""Trainium2 Bass kernel for nn_AdaptiveGraphGenerator (8-core SPMD).

Math (from the reference):
    node_feats = GELU(LN(x @ W_enc1 + b_enc1)) @ W_enc2 + b_enc2       [B,N,dim]
    adj_matrix = (1.0 > threshold) broadcast to [B,N,N,1]
The edge-MLP in the reference is dead code: gumbel-softmax over a singleton
axis is identically 1.0, so the adjacency depends only on `threshold`.

Sharding: row-shard the N=1024 nodes across 8 cores (128 rows each).  Each
core computes its node_feats slab and writes its [128, 1024] all-ones(*mask)
adjacency slab.  No cross-core communication.
"""

import sys

if "/opt/trn_rl_repo" not in sys.path:
    sys.path.insert(0, "/opt/trn_rl_repo")

import numpy as np

from concourse import bacc, mybir, tile
from concourse.bass_utils import run_bass_kernel_spmd

N_CORES = 8
N = 1024
DIM = 128
HID = 2 * DIM
ROWS = N // N_CORES  # rows of the node set per core
F32 = mybir.dt.float32
LN_EPS = 1e-5
NEWTON_ITERS = 4  # rsqrt Newton iterations (seed 2/(1+v)); exact to <1e-7 for v in [0.1, 10]

AF = mybir.ActivationFunctionType
ALU = mybir.AluOpType

_CACHE = {}


def _build():
    nc = bacc.Bacc(None, target_bir_lowering=False)

    x_d = nc.declare_dram_parameter("x", [ROWS, DIM], F32, isOutput=False)
    w1_d = nc.declare_dram_parameter("w1", [DIM, HID], F32, isOutput=False)
    b1_d = nc.declare_dram_parameter("b1", [1, HID], F32, isOutput=False)
    lng_d = nc.declare_dram_parameter("lng", [1, HID], F32, isOutput=False)
    lnb_d = nc.declare_dram_parameter("lnb", [1, HID], F32, isOutput=False)
    w2_d = nc.declare_dram_parameter("w2", [HID, DIM], F32, isOutput=False)
    b2_d = nc.declare_dram_parameter("b2", [1, DIM], F32, isOutput=False)
    th_d = nc.declare_dram_parameter("th", [1, 1], F32, isOutput=False)
    nf_d = nc.declare_dram_parameter("nf", [ROWS, DIM], F32, isOutput=True)
    adj_d = nc.declare_dram_parameter("adj", [ROWS, N], F32, isOutput=True)

    ident_d = nc.inline_tensor(np.eye(DIM, dtype=np.float32), name="ident")

    with tile.TileContext(nc) as tc:
        with (
            tc.tile_pool(name="sb", bufs=1) as sb,
            tc.tile_pool(name="ps", bufs=1, space="PSUM") as ps,
        ):
            # constants
            ones_col = sb.tile([1, ROWS], F32)  # lhsT for K=1 broadcast matmuls
            nc.vector.memset(ones_col[:], 1.0)
            # warm up the gelu act-table early so the load overlaps input DMAs
            warm = sb.tile([1, 1], F32)
            nc.scalar.activation(warm[:], ones_col[0:1, 0:1], AF.Gelu)

            # input DMAs
            x_sb = sb.tile([ROWS, DIM], F32)
            nc.sync.dma_start(out=x_sb[:], in_=x_d[:])
            ident = sb.tile([DIM, DIM], F32)
            nc.sync.dma_start(out=ident[:], in_=ident_d[:])
            w1_sb = sb.tile([DIM, HID], F32)
            nc.sync.dma_start(out=w1_sb[:], in_=w1_d[:])
            b1_sb = sb.tile([1, HID], F32)
            nc.sync.dma_start(out=b1_sb[:], in_=b1_d[:])
            lng_sb = sb.tile([1, HID], F32)
            nc.sync.dma_start(out=lng_sb[:], in_=lng_d[:])
            lnb_sb = sb.tile([1, HID], F32)
            nc.sync.dma_start(out=lnb_sb[:], in_=lnb_d[:])
            w2a_sb = sb.tile([DIM, DIM], F32)
            nc.sync.dma_start(out=w2a_sb[:], in_=w2_d[0:DIM, :])
            w2b_sb = sb.tile([DIM, DIM], F32)
            nc.sync.dma_start(out=w2b_sb[:], in_=w2_d[DIM:HID, :])
            b2_sb = sb.tile([1, DIM], F32)
            nc.sync.dma_start(out=b2_sb[:], in_=b2_d[:])
            th_sb = sb.tile([1, 1], F32)
            nc.sync.dma_start(out=th_sb[:], in_=th_d[:])

            # ---- adjacency slab: all ones * (1 > threshold) ----
            sgn = sb.tile([1, 1], F32)
            nc.scalar.activation(sgn[:], th_sb[:], AF.Sign, bias=1.0, scale=-1.0)
            msk = sb.tile([1, 1], F32)
            nc.scalar.activation(msk[:], sgn[:], AF.Relu)
            mask_ps = ps.tile([ROWS, 1], F32)
            nc.tensor.matmul(mask_ps[:], ones_col[:], msk[:], start=True, stop=True)
            mask_col = sb.tile([ROWS, 1], F32)
            nc.any.tensor_copy(mask_col[:], mask_ps[:])
            adj_sb = sb.tile([ROWS, N], F32)
            nc.gpsimd.memset(adj_sb[:], 1.0)
            nc.vector.tensor_scalar_mul(adj_sb[:], adj_sb[:], mask_col[:])
            nc.sync.dma_start(out=adj_d[:], in_=adj_sb[:])

            # ---- node encoder ----
            # xT = x.T via PE transpose
            xT_ps = ps.tile([DIM, ROWS], F32)
            nc.tensor.transpose(xT_ps[:], x_sb[:], ident[:])
            xT_sb = sb.tile([DIM, ROWS], F32)
            nc.any.tensor_copy(xT_sb[:], xT_ps[:])

            # h1 = x @ W1 + b1
            h1_ps = ps.tile([ROWS, HID], F32)
            nc.tensor.matmul(h1_ps[:], xT_sb[:], w1_sb[:], start=True, stop=False)
            nc.tensor.matmul(h1_ps[:], ones_col[:], b1_sb[:], start=False, stop=True)

            # layernorm stats
            stats = sb.tile([ROWS, 6], F32)
            nc.vector.bn_stats(stats[:], h1_ps[:])
            mv = sb.tile([ROWS, 2], F32)
            nc.vector.bn_aggr(mv[:], stats[:])
            mean = mv[:, 0:1]
            var = mv[:, 1:2]

            # rstd = 1/sqrt(var+eps) via Newton on DVE (avoids a 2.7us ACT
            # sqrt-table swap; the only ACT table used is gelu_and_others)
            veps = sb.tile([ROWS, 1], F32)
            nc.vector.tensor_scalar_add(veps[:], var, LN_EPS)
            seed_a = sb.tile([ROWS, 1], F32)
            nc.vector.tensor_scalar_add(seed_a[:], veps[:], 1.0)
            seed_r = sb.tile([ROWS, 1], F32)
            nc.vector.reciprocal(seed_r[:], seed_a[:])
            y = sb.tile([ROWS, 1], F32)
            nc.vector.tensor_scalar_mul(y[:], seed_r[:], 2.0)
            t = sb.tile([ROWS, 1], F32)
            s = sb.tile([ROWS, 1], F32)
            for _ in range(NEWTON_ITERS):
                nc.vector.tensor_mul(t[:], y[:], y[:])
                nc.vector.tensor_mul(t[:], t[:], veps[:])
                nc.vector.tensor_scalar(s[:], t[:], -0.5, 1.5, ALU.mult, ALU.add)
                nc.vector.tensor_mul(y[:], y[:], s[:])

            # hn = (h1 - mean) * rstd * ln_g + ln_b
            hn = sb.tile([ROWS, HID], F32)
            nc.vector.tensor_scalar(hn[:], h1_ps[:], mean, y[:], ALU.subtract, ALU.mult)
            lng_ps = ps.tile([ROWS, HID], F32)
            nc.tensor.matmul(lng_ps[:], ones_col[:], lng_sb[:], start=True, stop=True)
            lnb_ps = ps.tile([ROWS, HID], F32)
            nc.tensor.matmul(lnb_ps[:], ones_col[:], lnb_sb[:], start=True, stop=True)
            nc.vector.tensor_mul(hn[:], hn[:], lng_ps[:])
            nc.vector.tensor_add(hn[:], hn[:], lnb_ps[:])

            # g = gelu(hn)  (exact-erf gelu table)
            g = sb.tile([ROWS, HID], F32)
            nc.scalar.activation(g[:], hn[:], AF.Gelu)

            # node_feats = g @ W2 + b2 ; contraction over HID=256 -> 2 K-blocks
            gT0_ps = ps.tile([DIM, ROWS], F32)
            nc.tensor.transpose(gT0_ps[:], g[:, 0:DIM], ident[:])
            gT1_ps = ps.tile([DIM, ROWS], F32)
            nc.tensor.transpose(gT1_ps[:], g[:, DIM:HID], ident[:])
            gT0 = sb.tile([DIM, ROWS], F32)
            nc.any.tensor_copy(gT0[:], gT0_ps[:])
            gT1 = sb.tile([DIM, ROWS], F32)
            nc.any.tensor_copy(gT1[:], gT1_ps[:])
            nf_ps = ps.tile([ROWS, DIM], F32)
            nc.tensor.matmul(nf_ps[:], gT0[:], w2a_sb[:], start=True, stop=False)
            nc.tensor.matmul(nf_ps[:], gT1[:], w2b_sb[:], start=False, stop=False)
            nc.tensor.matmul(nf_ps[:], ones_col[:], b2_sb[:], start=False, stop=True)
            nf_sb = sb.tile([ROWS, DIM], F32)
            nc.any.tensor_copy(nf_sb[:], nf_ps[:])
            nc.sync.dma_start(out=nf_d[:], in_=nf_sb[:])

    nc.finalize()
    return nc


def _get_nc():
    if "nc" not in _CACHE:
        _CACHE["nc"] = _build()
    return _CACHE["nc"]


def kernel(x, W_enc1, b_enc1, ln_g, ln_b, W_enc2, b_enc2,
           W_e1, b_e1, W_e2, b_e2, threshold, **_unused):
    nc = _get_nc()
    x = np.ascontiguousarray(np.asarray(x, dtype=np.float32))
    B, n, d = x.shape
    assert (B, n, d) == (1, N, DIM), (B, n, d)
    xf = x.reshape(N, DIM)
    shared = {
        "w1": np.ascontiguousarray(np.asarray(W_enc1, np.float32)),
        "b1": np.asarray(b_enc1, np.float32).reshape(1, HID),
        "lng": np.asarray(ln_g, np.float32).reshape(1, HID),
        "lnb": np.asarray(ln_b, np.float32).reshape(1, HID),
        "w2": np.ascontiguousarray(np.asarray(W_enc2, np.float32)),
        "b2": np.asarray(b_enc2, np.float32).reshape(1, DIM),
        "th": np.asarray(threshold, np.float32).reshape(1, 1),
    }
    in_maps = [
        {"x": np.ascontiguousarray(xf[c * ROWS:(c + 1) * ROWS]), **shared}
        for c in range(N_CORES)
    ]
    res = run_bass_kernel_spmd(nc, in_maps, core_ids=list(range(N_CORES))).results
    nf = np.concatenate([res[c]["nf"] for c in range(N_CORES)], axis=0)
    adj = np.concatenate([res[c]["adj"] for c in range(N_CORES)], axis=0)
    return adj.reshape(B, N, N, 1), nf.reshape(B, N, DIM)


# revision 3
# speedup vs baseline: 1.0986x; 1.0986x over previous
"""Trainium2 Bass kernel for nn_AdaptiveGraphGenerator (8-core SPMD).

Math (from the reference):
    node_feats = GELU(LN(x @ W_enc1 + b_enc1)) @ W_enc2 + b_enc2       [B,N,dim]
    adj_matrix = (1.0 > threshold) broadcast to [B,N,N,1]
The edge-MLP in the reference is dead code: gumbel-softmax over a singleton
axis is identically 1.0, so the adjacency depends only on `threshold`.

Sharding: row-shard the N=1024 nodes across 8 cores (128 rows each).  Each
core computes its node_feats slab and writes its [128, 1024] adjacency slab.
No cross-core communication.

Inputs are packed host-side into three DRAM parameters to avoid serializing
ten tiny DMAs on one HWDGE queue:
    xp [128, 384] per-core : x_shard(128) | W_enc1(256)
    wp [128, 384] shared   : ident(128) | W_enc2[0:128](128) | W_enc2[128:](128)
    sp [1, 897]   shared   : th(1) | b1(256) | ln_g(256) | ln_b(256) | b2(128)
"""

import sys

if "/opt/trn_rl_repo" not in sys.path:
    sys.path.insert(0, "/opt/trn_rl_repo")

import numpy as np

from concourse import bacc, mybir, tile
from concourse.bass_utils import run_bass_kernel_spmd

N_CORES = 8
N = 1024
DIM = 128
HID = 2 * DIM
ROWS = N // N_CORES
F32 = mybir.dt.float32
LN_EPS = 1e-5
NEWTON_ITERS = 2  # rsqrt Newton iterations (seed 2/(1+v))

AF = mybir.ActivationFunctionType
ALU = mybir.AluOpType

_CACHE = {}


def _build():
    nc = bacc.Bacc(None, target_bir_lowering=False)

    xp_d = nc.declare_dram_parameter("xp", [ROWS, DIM + HID], F32, isOutput=False)
    wp_d = nc.declare_dram_parameter("wp", [DIM, 3 * DIM], F32, isOutput=False)
    sp_d = nc.declare_dram_parameter("sp", [1, 1 + 3 * HID + DIM], F32, isOutput=False)
    nf_d = nc.declare_dram_parameter("nf", [ROWS, DIM], F32, isOutput=True)
    adj_d = nc.declare_dram_parameter("adj", [ROWS, N], F32, isOutput=True)

    with tile.TileContext(nc) as tc:
        with (
            tc.tile_pool(name="sb", bufs=1) as sb,
            tc.tile_pool(name="ps", bufs=1, space="PSUM") as ps,
        ):
            # constants
            ones_col = sb.tile([1, ROWS], F32)
            nc.vector.memset(ones_col[:], 1.0)
            # warm up the gelu act-table early so the load overlaps input DMAs
            warm = sb.tile([1, 1], F32)
            nc.scalar.activation(warm[:], ones_col[0:1, 0:1], AF.Gelu)

            # ---- input DMAs: sp + wp on sync queue, xp on scalar queue ----
            sp_sb = sb.tile([1, 1 + 3 * HID + DIM], F32)
            nc.sync.dma_start(out=sp_sb[:], in_=sp_d[:])
            th = sp_sb[:, 0:1]
            b1 = sp_sb[:, 1:1 + HID]
            lng = sp_sb[:, 1 + HID:1 + 2 * HID]
            lnb = sp_sb[:, 1 + 2 * HID:1 + 3 * HID]
            b2 = sp_sb[:, 1 + 3 * HID:1 + 3 * HID + DIM]

            xp_sb = sb.tile([ROWS, DIM + HID], F32)
            nc.scalar.dma_start(out=xp_sb[:], in_=xp_d[:])
            x_sb = xp_sb[:, 0:DIM]
            w1_sb = xp_sb[:, DIM:DIM + HID]

            wp_sb = sb.tile([DIM, 3 * DIM], F32)
            nc.sync.dma_start(out=wp_sb[:], in_=wp_d[:])
            ident = wp_sb[:, 0:DIM]
            w2a = wp_sb[:, DIM:2 * DIM]
            w2b = wp_sb[:, 2 * DIM:3 * DIM]

            # ---- adjacency slab: ones * (1 > threshold), off critical path ----
            sgn = sb.tile([1, 1], F32)
            nc.scalar.activation(sgn[:], th, AF.Sign, bias=1.0, scale=-1.0)
            msk = sb.tile([1, 1], F32)
            nc.scalar.activation(msk[:], sgn[:], AF.Relu)
            mask_ps = ps.tile([ROWS, 1], F32)
            nc.tensor.matmul(mask_ps[:], ones_col[:], msk[:], start=True, stop=True)
            mask_col = sb.tile([ROWS, 1], F32)
            nc.vector.tensor_copy(mask_col[:], mask_ps[:])
            adj_sb = sb.tile([ROWS, N], F32)
            nc.gpsimd.memset(adj_sb[:], 1.0)
            nc.scalar.activation(adj_sb[:], adj_sb[:], AF.Copy, bias=0.0,
                                 scale=mask_col[:])
            nc.scalar.dma_start(out=adj_d[:], in_=adj_sb[:])

            # ---- node encoder ----
            xT_ps = ps.tile([DIM, ROWS], F32)
            nc.tensor.transpose(xT_ps[:], x_sb, ident)
            xT_sb = sb.tile([DIM, ROWS], F32)
            nc.vector.tensor_copy(xT_sb[:], xT_ps[:])

            h1_ps = ps.tile([ROWS, HID], F32)
            nc.tensor.matmul(h1_ps[:], xT_sb[:], w1_sb, start=True, stop=False)
            nc.tensor.matmul(h1_ps[:], ones_col[:], b1, start=False, stop=True)

            # LN stats
            stats = sb.tile([ROWS, 6], F32)
            nc.vector.bn_stats(stats[:], h1_ps[:])
            mv = sb.tile([ROWS, 2], F32)
            nc.vector.bn_aggr(mv[:], stats[:])
            mean = mv[:, 0:1]
            var = mv[:, 1:2]

            # rstd = 1/sqrt(var+eps): Newton on DVE (no ACT sqrt-table swap)
            veps = sb.tile([ROWS, 1], F32)
            nc.vector.tensor_scalar_add(veps[:], var, LN_EPS)
            seed_a = sb.tile([ROWS, 1], F32)
            nc.vector.tensor_scalar_add(seed_a[:], var, 1.0 + LN_EPS)
            seed_r = sb.tile([ROWS, 1], F32)
            nc.vector.reciprocal(seed_r[:], seed_a[:])
            y = sb.tile([ROWS, 1], F32)
            nc.vector.tensor_scalar_mul(y[:], seed_r[:], 2.0)
            t = sb.tile([ROWS, 1], F32)
            s = sb.tile([ROWS, 1], F32)
            for _ in range(NEWTON_ITERS):
                # t = v * y^2 ; s = 1.5 - 0.5 t ; y *= s
                nc.vector.tensor_scalar(t[:], y[:], y[:], veps[:], ALU.mult, ALU.mult)
                nc.vector.tensor_scalar(s[:], t[:], -0.5, 1.5, ALU.mult, ALU.add)
                nc.vector.tensor_scalar(y[:], y[:], s[:], None, ALU.mult)

            # ln_g / ln_b broadcast to [ROWS, HID] via stride-0 DMA from DRAM
            lng_bc = sb.tile([ROWS, HID], F32)
            nc.sync.dma_start(
                out=lng_bc[:],
                in_=sp_d[:, 1 + HID:1 + 2 * HID].broadcast_to([ROWS, HID]),
            )
            lnb_bc = sb.tile([ROWS, HID], F32)
            nc.sync.dma_start(
                out=lnb_bc[:],
                in_=sp_d[:, 1 + 2 * HID:1 + 3 * HID].broadcast_to([ROWS, HID]),
            )

            # hn = ((h1 - mean) * ln_g) * rstd + ln_b   (rstd commutes with ln_g)
            hn = sb.tile([ROWS, HID], F32)
            nc.vector.scalar_tensor_tensor(hn[:], h1_ps[:], mean, lng_bc[:],
                                           ALU.subtract, ALU.mult)
            nc.vector.scalar_tensor_tensor(hn[:], hn[:], y[:], lnb_bc[:],
                                           ALU.mult, ALU.add)

            # g = gelu(hn)
            g = sb.tile([ROWS, HID], F32)
            nc.scalar.activation(g[:], hn[:], AF.Gelu)

            # nf = g @ W2 + b2
            gT0_ps = ps.tile([DIM, ROWS], F32)
            nc.tensor.transpose(gT0_ps[:], g[:, 0:DIM], ident)
            gT1_ps = ps.tile([DIM, ROWS], F32)
            nc.tensor.transpose(gT1_ps[:], g[:, DIM:HID], ident)
            gT0 = sb.tile([DIM, ROWS], F32)
            nc.vector.tensor_copy(gT0[:], gT0_ps[:])
            gT1 = sb.tile([DIM, ROWS], F32)
            nc.vector.tensor_copy(gT1[:], gT1_ps[:])
            nf_ps = ps.tile([ROWS, DIM], F32)
            nc.tensor.matmul(nf_ps[:], gT0[:], w2a, start=True, stop=False)
            nc.tensor.matmul(nf_ps[:], gT1[:], w2b, start=False, stop=False)
            nc.tensor.matmul(nf_ps[:], ones_col[:], b2, start=False, stop=True)
            nf_sb = sb.tile([ROWS, DIM], F32)
            nc.scalar.copy(nf_sb[:], nf_ps[:])
            nc.sync.dma_start(out=nf_d[:], in_=nf_sb[:])

    nc.finalize()
    return nc


def _get_nc():
    if "nc" not in _CACHE:
        _CACHE["nc"] = _build()
    return _CACHE["nc"]


def _pack_inputs(x, W_enc1, b_enc1, ln_g, ln_b, W_enc2, b_enc2, threshold):
    xf = np.asarray(x, np.float32).reshape(N, DIM)
    w1 = np.asarray(W_enc1, np.float32)
    w2 = np.asarray(W_enc2, np.float32)
    wp = np.concatenate(
        [np.eye(DIM, dtype=np.float32), w2[0:DIM], w2[DIM:HID]], axis=1
    )
    sp = np.concatenate(
        [np.asarray(threshold, np.float32).reshape(1),
         np.asarray(b_enc1, np.float32).reshape(HID),
         np.asarray(ln_g, np.float32).reshape(HID),
         np.asarray(ln_b, np.float32).reshape(HID),
         np.asarray(b_enc2, np.float32).reshape(DIM)]
    ).reshape(1, -1)
    wp = np.ascontiguousarray(wp)
    sp = np.ascontiguousarray(sp)
    in_maps = []
    for c in range(N_CORES):
        xp = np.ascontiguousarray(
            np.concatenate([xf[c * ROWS:(c + 1) * ROWS], w1], axis=1)
        )
        in_maps.append({"xp": xp, "wp": wp, "sp": sp})
    return in_maps


def kernel(x, W_enc1, b_enc1, ln_g, ln_b, W_enc2, b_enc2,
           W_e1, b_e1, W_e2, b_e2, threshold, **_unused):
    nc = _get_nc()
    B = np.asarray(x).shape[0]
    in_maps = _pack_inputs(x, W_enc1, b_enc1, ln_g, ln_b, W_enc2, b_enc2,
                           threshold)
    res = run_bass_kernel_spmd(nc, in_maps, core_ids=list(range(N_CORES))).results
    nf = np.concatenate([res[c]["nf"] for c in range(N_CORES)], axis=0)
    adj = np.concatenate([res[c]["adj"] for c in range(N_CORES)], axis=0)
    return adj.reshape(B, N, N, 1), nf.reshape(B, N, DIM)
